# revision 1
# baseline (speedup 1.0000x reference)
"""GCN VGAE encoder (2-layer, mu/logstd heads) on 8 Trainium2 NeuronCores.

Strategy (edge-parallel over dst-sorted CSR):
  - Host: sort edges by dst; bucket per (128-node dst block j, source quarter
    g4 = src//25088); pad each (j,g4) run to a uniform (across cores) chunk
    count of 128 edges; emit int16 gather rows (sub-table local), local dst
    offsets, and the wrapped index layout dma_gather wants.  Degrees come
    free from the CSR build.  Pad edges use dstl=128 -> all-zero one-hot
    column -> contribute nothing.
  - Device (SPMD x8, each core owns 98 node blocks = 12544 nodes):
      disv = 1/sqrt(deg+1); y1 = disv*x written to a fat (64-wide) table
      L1: dma_gather y1[src] per 128-edge chunk (4 SWDGE queues), one-hot
          via DVE is_equal, matmul-accumulate transposed sums in PSUM per
          block, self-loop added via accumulating PE transpose;
          h = relu(disv*(s1@W1)+b1); y2 = disv*h
      AllGather y2 shards -> replicated table (same row map as y1 table,
      so one index array serves both layers)
      L2: same gather/one-hot/matmul over y2, self-loop via transpose,
          heads mu/logstd = disv*(s2@W) + b via small matmuls.
All floating-point math runs on device; the host only reorders integers.
"""
import math

import numpy as np

import concourse.bass as bass
import concourse.bacc as bacc
import concourse.mybir as mybir
import concourse.tile as tile
from concourse.bass_utils import run_bass_kernel_spmd
from concourse.masks import make_identity

P = 128
N_CORES = 8
NG = 4                      # source-quarter groups (int16 sub-tables)
F32 = mybir.dt.float32
I32 = mybir.dt.int32
I16 = mybir.dt.int16

_CACHE = {}


# ---------------------------------------------------------------- host prep
def _prep(x, edge_index):
    N = x.shape[0]
    in_ch = x.shape[1]
    nbc = math.ceil(math.ceil(N / N_CORES) / P)      # blocks per core (98)
    npc = nbc * P                                    # nodes per core (12544)
    npad = N_CORES * npc                             # padded nodes (100352)
    nblk = N_CORES * nbc                             # blocks (784)
    nsub = npad // NG                                # sub-table rows (25088)

    src = np.asarray(edge_index[0]).astype(np.int64)
    dst = np.asarray(edge_index[1]).astype(np.int64)
    E = src.shape[0]

    deg = np.bincount(dst, minlength=npad).astype(np.int32)

    # source quarter from the SHARD of the source node (c//2), which the
    # block permutation does not change; the within-sub-table row uses the
    # permuted position and is computed after `pos` exists.
    c_ = src // npc
    rem = src - c_ * npc
    g4 = c_ // 2                                      # source quarter

    # sort edges by (dst-block, g4, dst)
    order = np.argsort(((dst >> 7) * NG + g4) * 128 + (dst & 127),
                       kind="stable")
    dst_s = dst[order]
    src_s = src[order]

    cd = dst_s // npc
    j_s = (dst_s - cd * npc) >> 7                     # block pos within core
    # run id: (core, block, g4)
    run = (cd * nbc + j_s) * NG + g4[order]
    nrun = nblk * NG
    counts = np.bincount(run, minlength=nrun)
    rstart = np.zeros(nrun + 1, np.int64)
    np.cumsum(counts, out=rstart[1:])

    # chunks per (j, g4): max over cores (SPMD-uniform).  Each core may
    # process its blocks in any order; sorting by size aligns big blocks
    # with big blocks across cores and shrinks the max-over-cores padding.
    cnt = counts.reshape(N_CORES, nbc, NG)
    perm = np.argsort(-cnt.sum(axis=2), axis=1)                # [NC, nbc]
    pos = np.empty_like(perm)
    for c in range(N_CORES):
        pos[c, perm[c]] = np.arange(nbc)
    cntp = np.take_along_axis(cnt, perm[:, :, None], axis=1)
    S = np.ceil(cntp / P).astype(np.int64).max(axis=0)         # [nbc, NG]
    Tj = S.sum(axis=1)                                         # chunks per block
    off = np.zeros(nbc + 1, np.int64)
    np.cumsum(Tj, out=off[1:])
    T = int(off[-1])
    # column index of (j, g4, t):  off[j] + sum_{g<g4} S[j,g] + t
    goff = np.zeros((nbc, NG), np.int64)
    goff[:, 1:] = np.cumsum(S[:, :-1], axis=1)
    # per-g4 chunk counters (j-major) for gather/idx layout
    Tg = S.sum(axis=0)                                         # chunks per g4
    cidx0 = np.zeros((nbc, NG), np.int64)                      # running base
    cidx0[1:, :] = np.cumsum(S[:-1, :], axis=0)

    posg = pos.reshape(-1)                            # position of global blk
    cs = src_s // npc
    rems = src_s - cs * npc
    rows = cs * npc + (rems & 127) * nbc + posg[src_s >> 7]
    rloc_s = (rows - (cs // 2) * nsub).astype(np.int64)

    dstl = np.full((N_CORES, P, T), 128.0, np.float32)         # 128 => no-op
    idxw = [np.zeros((N_CORES, P, int(Tg[g]) * 8), np.int16) for g in range(NG)]

    for c in range(N_CORES):
        srcl_g = [np.zeros((P, int(Tg[g])), np.int16) for g in range(NG)]
        for j in range(nbc):
            for g in range(NG):
                r = (c * nbc + int(perm[c, j])) * NG + g
                e0, e1 = rstart[r], rstart[r + 1]
                n_e = e1 - e0
                if n_e == 0:
                    continue
                i = np.arange(n_e)
                lane = i & 127
                col = off[j] + goff[j, g] + (i >> 7)
                dstl[c, lane, col] = (dst_s[e0:e1] & 127).astype(np.float32)
                ccol = cidx0[j, g] + (i >> 7)
                srcl_g[g][lane, ccol] = rloc_s[e0:e1].astype(np.int16)
        for g in range(NG):
            flat = srcl_g[g].T.ravel()                # i = chunk*128 + lane
            w16 = flat.reshape(-1, 16).T              # [16, Tg*8]
            idxw[g][c] = np.tile(w16, (8, 1))         # replicate to 128 parts

    # replicated node-space arrays (permuted block order, partition-inner)
    gidx = np.concatenate([c * nbc + perm[c] for c in range(N_CORES)])
    xpad = np.zeros((npad, in_ch), np.float32)
    xpad[:N] = np.asarray(x, np.float32)
    x_g = xpad.reshape(nblk, P, in_ch)[gidx].transpose(1, 0, 2).reshape(P, -1)
    deg_g = deg.reshape(nblk, P)[gidx].T.copy()
    # per-core own slices
    x_own = np.stack([
        x_g[:, c * nbc * in_ch:(c + 1) * nbc * in_ch] for c in range(N_CORES)])
    deg_own = np.stack([deg_g[:, c * nbc:(c + 1) * nbc] for c in range(N_CORES)])

    iota = np.tile(np.arange(P, dtype=np.float32), (P, 1))

    meta = dict(N=N, E=E, in_ch=in_ch, nbc=nbc, npc=npc, npad=npad,
                nblk=nblk, nsub=nsub, T=T,
                S=tuple(map(tuple, S.tolist())), perm=perm,
                off=off, goff=goff, cidx0=cidx0, Tg=tuple(int(t) for t in Tg))
    arrays = dict(dstl=dstl, idxw=idxw, x_g=x_g, deg_g=deg_g,
                  x_own=x_own, deg_own=deg_own, iota=iota)
    return meta, arrays


# ---------------------------------------------------------------- device build
def _build(meta, in_ch, hid, out_ch):
    nbc, nblk, T = meta["nbc"], meta["nblk"], meta["T"]
    npc, nsub = meta["npc"], meta["nsub"]
    S, off, goff, cidx0, Tg = (meta["S"], meta["off"], meta["goff"],
                               meta["cidx0"], meta["Tg"])
    SPC = 14                 # chunks per gather instruction

    nc = bacc.Bacc("TRN2", target_bir_lowering=False, debug=False,
                   num_devices=N_CORES, num_swdge_queues=NG)

    x_g_d = nc.dram_tensor("x_g", [P, nblk * in_ch], F32, kind="ExternalInput")
    deg_g_d = nc.dram_tensor("deg_g", [P, nblk], I32, kind="ExternalInput")
    x_o_d = nc.dram_tensor("x_own", [P, nbc * in_ch], F32, kind="ExternalInput")
    deg_o_d = nc.dram_tensor("deg_own", [P, nbc], I32, kind="ExternalInput")
    dstl_d = nc.dram_tensor("dstl", [P, T], F32, kind="ExternalInput")
    idxw_d = [nc.dram_tensor(f"idxw{g}", [P, Tg[g] * 8], I16,
                             kind="ExternalInput") for g in range(NG)]
    iota_d = nc.dram_tensor("iota", [P, P], F32, kind="ExternalInput")
    w1_d = nc.dram_tensor("w1", [in_ch, hid], F32, kind="ExternalInput")
    b1_d = nc.dram_tensor("b1", [P, hid], F32, kind="ExternalInput")
    wmu_d = nc.dram_tensor("wmu", [hid, out_ch], F32, kind="ExternalInput")
    wls_d = nc.dram_tensor("wls", [hid, out_ch], F32, kind="ExternalInput")
    bmu_d = nc.dram_tensor("bmu", [P, out_ch], F32, kind="ExternalInput")
    bls_d = nc.dram_tensor("bls", [P, out_ch], F32, kind="ExternalInput")
    mu_o = nc.dram_tensor("mu_o", [P, nbc * out_ch], F32, kind="ExternalOutput")
    ls_o = nc.dram_tensor("ls_o", [P, nbc * out_ch], F32, kind="ExternalOutput")

    y1tab = [nc.dram_tensor(f"y1tab{g}", [nsub, hid], F32, kind="Internal")
             for g in range(NG)]
    cc_in = nc.dram_tensor("cc_in", [npc, hid], F32, kind="Internal")
    cc_out = nc.dram_tensor("cc_out", [N_CORES * npc, hid], F32,
                            kind="Internal", addr_space="Shared")

    # gather instruction spans per g4: list of (c0, n) chunk ranges
    spans = []
    for g in range(NG):
        sp = []
        for c0 in range(0, Tg[g], SPC):
            sp.append((c0, min(SPC, Tg[g] - c0)))
        spans.append(sp)

    with tile.TileContext(nc) as tc:
        with tc.tile_pool(name="const", bufs=1) as cp:
            dstl_t = cp.tile([P, T], F32)
            idxw_t = [cp.tile([P, Tg[g] * 8], I16, name=f"idxw{g}_t") for g in range(NG)]
            iota_t = cp.tile([P, P], F32)
            x_gt = cp.tile([P, nblk * in_ch], F32)
            deg_gt = cp.tile([P, nblk], I32)
            x_ot = cp.tile([P, nbc * in_ch], F32)
            deg_ot = cp.tile([P, nbc], I32)
            w1_t = cp.tile([in_ch, hid], F32)
            b1_t = cp.tile([P, hid], F32)
            wmu_t = cp.tile([hid, out_ch], F32)
            wls_t = cp.tile([hid, out_ch], F32)
            bmu_t = cp.tile([P, out_ch], F32)
            bls_t = cp.tile([P, out_ch], F32)
            ident = cp.tile([P, P], F32)
            disv_g = cp.tile([P, nblk], F32)
            disv_o = cp.tile([P, nbc], F32)
            y1_g = cp.tile([P, nblk * in_ch], F32)
            y1_own = cp.tile([P, nbc * in_ch], F32)
            y2_t = cp.tile([P, nbc * hid], F32)
            mu_sb = cp.tile([P, nbc * out_ch], F32)
            ls_sb = cp.tile([P, nbc * out_ch], F32)

            for dt_, tt in ((x_g_d, x_gt), (deg_g_d, deg_gt), (x_o_d, x_ot),
                            (deg_o_d, deg_ot), (dstl_d, dstl_t),
                            (iota_d, iota_t), (w1_d, w1_t), (b1_d, b1_t),
                            (wmu_d, wmu_t), (wls_d, wls_t), (bmu_d, bmu_t),
                            (bls_d, bls_t)):
                nc.sync.dma_start(out=tt[:], in_=dt_[:])
            for g in range(NG):
                nc.sync.dma_start(out=idxw_t[g][:], in_=idxw_d[g][:])
            make_identity(nc, ident[:])

            # disv = 1/sqrt(deg+1) (global + own)
            for deg_t, dv, n in ((deg_gt, disv_g, nblk), (deg_ot, disv_o, nbc)):
                nc.vector.tensor_copy(out=dv[:], in_=deg_t[:])
                nc.scalar.activation(dv[:], dv[:],
                                     mybir.ActivationFunctionType.Sqrt,
                                     bias=1.0)
                nc.vector.reciprocal(out=dv[:], in_=dv[:])

            # y1 = disv * x  (global + own)
            for xt, dv, yt, n in ((x_gt, disv_g, y1_g, nblk),
                                  (x_ot, disv_o, y1_own, nbc)):
                nc.vector.tensor_tensor(
                    out=yt[:].rearrange("p (b c) -> p b c", c=in_ch),
                    in0=xt[:].rearrange("p (b c) -> p b c", c=in_ch),
                    in1=dv[:].rearrange("p (b o) -> p b o", o=1)
                        .to_broadcast([P, n, in_ch]),
                    op=mybir.AluOpType.mult)

            # write fat y1 table: rows r = c*npc + p*nbc + bl, 64-wide
            with tc.tile_pool(name="stage", bufs=2) as stp:
                for c in range(N_CORES):
                    st = stp.tile([P, nbc * hid], F32, tag="st")
                    # fat columns 2:64 are never read (matmul slices 0:2),
                    # so no zero-fill is needed
                    nc.vector.tensor_copy(
                        out=st[:].rearrange("p (b c) -> p b c", c=hid)[:, :, :in_ch],
                        in_=y1_g[:].rearrange("p (b c) -> p b c", c=in_ch)
                            [:, c * nbc:(c + 1) * nbc, :])
                    nc.scalar.dma_start(
                        out=y1tab[c // 2].ap().rearrange(
                            "(h p b) w -> h p (b w)", h=2, p=P)[c % 2],
                        in_=st[:])

            # ---------------- Layer 1 ----------------
            with (
                tc.tile_pool(name="g1", bufs=16) as g1p,
                tc.tile_pool(name="oh1", bufs=3) as ohp,
                tc.tile_pool(name="s1", bufs=4) as s1p,
                tc.tile_pool(name="ps1", bufs=4, space="PSUM") as pp1,
                tc.tile_pool(name="psh", bufs=4, space="PSUM") as pph,
            ):
                gtiles = {g: [] for g in range(NG)}
                order1 = [(g, sp) for i in range(max(len(s) for s in spans))
                          for g in range(NG) for sp in spans[g][i:i + 1]]
                for g, (c0, n) in order1:
                        gt = g1p.tile([P, n * hid], F32, tag="g1")
                        nc.gpsimd.dma_gather(
                            out_ap=gt[:].rearrange("p (n c) -> p n c", c=hid),
                            in_ap=y1tab[g].ap()[:, :],
                            idxs_ap=idxw_t[g][:, c0 * 8:(c0 + n) * 8],
                            num_idxs=n * P, num_idxs_reg=n * P,
                            elem_size=hid, single_packet=False, queue_num=g)
                        gtiles[g].append((c0, n, gt))

                def msg(g, cidx):
                    for c0, n, gt in gtiles[g]:
                        if c0 <= cidx < c0 + n:
                            return gt, cidx - c0
                    raise AssertionError

                for j in range(nbc):
                    tj = int(off[j + 1] - off[j])
                    c0 = int(off[j])
                    oh = ohp.tile([P, tj * P], F32, tag="oh")
                    nc.vector.tensor_tensor(
                        out=oh[:].rearrange("p (s f) -> p s f", f=P),
                        in0=iota_t[:].rearrange("p (o f) -> p o f", o=1)
                            .to_broadcast([P, tj, P]),
                        in1=dstl_t[:, c0:c0 + tj]
                            .rearrange("p (s o) -> p s o", o=1)
                            .to_broadcast([P, tj, P]),
                        op=mybir.AluOpType.is_equal)
                    ps1 = pp1.tile([in_ch, P], F32, tag="ps1")
                    first = True
                    for g in range(NG):
                        for t in range(S[j][g]):
                            gt, k = msg(g, int(cidx0[j][g]) + t)
                            col = int(goff[j][g]) + t
                            nc.tensor.matmul(
                                out=ps1[:],
                                lhsT=gt[:, k * hid:k * hid + in_ch],
                                rhs=oh[:, col * P:(col + 1) * P],
                                start=first, stop=False)
                            first = False
                    nc.tensor.matmul(
                        out=ps1[:], lhsT=y1_own[:, j * in_ch:(j + 1) * in_ch],
                        rhs=ident[:], is_transpose=True,
                        start=first, stop=True)
                    s1T = s1p.tile([in_ch, P], F32, tag="s1T")
                    nc.vector.tensor_copy(out=s1T[:], in_=ps1[:])
                    ph = pph.tile([P, hid], F32, tag="ph")
                    nc.tensor.matmul(out=ph[:], lhsT=s1T[:], rhs=w1_t[:],
                                     start=True, stop=True)
                    nc.vector.tensor_tensor(
                        out=y2_t[:, j * hid:(j + 1) * hid],
                        in0=ph[:],
                        in1=disv_o[:, j:j + 1].to_broadcast([P, hid]),
                        op=mybir.AluOpType.mult)

            # h = relu(. + b1); y2 = disv * h
            y2_3 = y2_t[:].rearrange("p (b c) -> p b c", c=hid)
            nc.vector.tensor_tensor(
                out=y2_3, in0=y2_3,
                in1=b1_t[:].rearrange("p (o c) -> p o c", o=1)
                    .to_broadcast([P, nbc, hid]),
                op=mybir.AluOpType.add)
            nc.scalar.activation(y2_t[:], y2_t[:],
                                 mybir.ActivationFunctionType.Relu)
            nc.vector.tensor_tensor(
                out=y2_3, in0=y2_3,
                in1=disv_o[:].rearrange("p (b o) -> p b o", o=1)
                    .to_broadcast([P, nbc, hid]),
                op=mybir.AluOpType.mult)
            nc.sync.dma_start(
                out=cc_in.ap().rearrange("(p b) c -> p (b c)", p=P),
                in_=y2_t[:])
            nc.gpsimd.collective_compute(
                "AllGather", mybir.AluOpType.bypass,
                ins=[cc_in[:]], outs=[cc_out[:]],
                replica_groups=[list(range(N_CORES))])

            # ---------------- Layer 2 ----------------
            with (
                tc.tile_pool(name="g2", bufs=16) as g2p,
                tc.tile_pool(name="oh2", bufs=3) as ohp2,
                tc.tile_pool(name="s2", bufs=4) as s2p,
                tc.tile_pool(name="ps2", bufs=3, space="PSUM") as pp2,
                tc.tile_pool(name="pmu", bufs=2, space="PSUM") as ppm,
            ):
                gtiles2 = {g: [] for g in range(NG)}
                for g, (c0, n) in order1:
                        gt = g2p.tile([P, n * hid], F32, tag="g2")
                        nc.gpsimd.dma_gather(
                            out_ap=gt[:].rearrange("p (n c) -> p n c", c=hid),
                            in_ap=cc_out.ap()[g * nsub:(g + 1) * nsub, :],
                            idxs_ap=idxw_t[g][:, c0 * 8:(c0 + n) * 8],
                            num_idxs=n * P, num_idxs_reg=n * P,
                            elem_size=hid, single_packet=False, queue_num=g)
                        gtiles2[g].append((c0, n, gt))

                def msg2(g, cidx):
                    for c0, n, gt in gtiles2[g]:
                        if c0 <= cidx < c0 + n:
                            return gt, cidx - c0
                    raise AssertionError

                for j in range(nbc):
                    tj = int(off[j + 1] - off[j])
                    c0 = int(off[j])
                    oh = ohp2.tile([P, tj * P], F32, tag="oh2")
                    nc.vector.tensor_tensor(
                        out=oh[:].rearrange("p (s f) -> p s f", f=P),
                        in0=iota_t[:].rearrange("p (o f) -> p o f", o=1)
                            .to_broadcast([P, tj, P]),
                        in1=dstl_t[:, c0:c0 + tj]
                            .rearrange("p (s o) -> p s o", o=1)
                            .to_broadcast([P, tj, P]),
                        op=mybir.AluOpType.is_equal)
                    ps2 = pp2.tile([hid, P], F32, tag="ps2")
                    first = True
                    for g in range(NG):
                        for t in range(S[j][g]):
                            gt, k = msg2(g, int(cidx0[j][g]) + t)
                            col = int(goff[j][g]) + t
                            nc.tensor.matmul(
                                out=ps2[:],
                                lhsT=gt[:, k * hid:(k + 1) * hid],
                                rhs=oh[:, col * P:(col + 1) * P],
                                start=first, stop=False)
                            first = False
                    nc.tensor.matmul(
                        out=ps2[:], lhsT=y2_t[:, j * hid:(j + 1) * hid],
                        rhs=ident[:], is_transpose=True,
                        start=first, stop=True)
                    s2T = s2p.tile([hid, P], F32, tag="s2T")
                    nc.vector.tensor_copy(out=s2T[:], in_=ps2[:])
                    pm = ppm.tile([P, 2 * out_ch], F32, tag="pm")
                    nc.tensor.matmul(out=pm[:, :out_ch], lhsT=s2T[:],
                                     rhs=wmu_t[:], start=True, stop=True)
                    nc.tensor.matmul(out=pm[:, out_ch:], lhsT=s2T[:],
                                     rhs=wls_t[:], start=True, stop=True)
                    nc.scalar.activation(
                        mu_sb[:, j * out_ch:(j + 1) * out_ch], pm[:, :out_ch],
                        mybir.ActivationFunctionType.Copy,
                        scale=disv_o[:, j:j + 1])
                    nc.scalar.activation(
                        ls_sb[:, j * out_ch:(j + 1) * out_ch], pm[:, out_ch:],
                        mybir.ActivationFunctionType.Copy,
                        scale=disv_o[:, j:j + 1])

            for sb, bt, ot in ((mu_sb, bmu_t, mu_o), (ls_sb, bls_t, ls_o)):
                nc.vector.tensor_tensor(
                    out=sb[:].rearrange("p (b c) -> p b c", c=out_ch),
                    in0=sb[:].rearrange("p (b c) -> p b c", c=out_ch),
                    in1=bt[:].rearrange("p (o c) -> p o c", o=1)
                        .to_broadcast([P, nbc, out_ch]),
                    op=mybir.AluOpType.add)
                nc.sync.dma_start(out=ot[:], in_=sb[:])

    nc.compile()
    return nc


# ---------------------------------------------------------------- entry point
def kernel(x, edge_index, W1, b1, W_mu, b_mu, W_logstd, b_logstd,
           _want_results=False, _run_kwargs=None):
    x = np.asarray(x, np.float32)
    in_ch, hid, out_ch = W1.shape[0], W1.shape[1], W_mu.shape[1]
    meta, arrays = _prep(x, edge_index)

    key = (meta["N"], meta["E"], in_ch, hid, out_ch, meta["S"])
    if key not in _CACHE:
        _CACHE[key] = _build(meta, in_ch, hid, out_ch)
    nc = _CACHE[key]

    b1_b = np.tile(np.asarray(b1, np.float32), (P, 1))
    bmu_b = np.tile(np.asarray(b_mu, np.float32), (P, 1))
    bls_b = np.tile(np.asarray(b_logstd, np.float32), (P, 1))

    in_maps = []
    for c in range(N_CORES):
        m = dict(
            x_g=arrays["x_g"], deg_g=arrays["deg_g"],
            x_own=arrays["x_own"][c], deg_own=arrays["deg_own"][c],
            dstl=arrays["dstl"][c], iota=arrays["iota"],
            w1=np.asarray(W1, np.float32), b1=b1_b,
            wmu=np.asarray(W_mu, np.float32), wls=np.asarray(W_logstd, np.float32),
            bmu=bmu_b, bls=bls_b)
        for g in range(NG):
            m[f"idxw{g}"] = arrays["idxw"][g][c]
        in_maps.append(m)

    res = run_bass_kernel_spmd(nc, in_maps, core_ids=list(range(N_CORES)),
                               **(_run_kwargs or {}))

    N, nbc, npc = meta["N"], meta["nbc"], meta["npc"]
    perm = meta["perm"]
    mu = np.empty((N_CORES * npc, out_ch), np.float32)
    ls = np.empty((N_CORES * npc, out_ch), np.float32)
    for c in range(N_CORES):
        mo = res.results[c]["mu_o"].reshape(P, nbc, out_ch).transpose(1, 0, 2)
        lo = res.results[c]["ls_o"].reshape(P, nbc, out_ch).transpose(1, 0, 2)
        blk = c * npc + perm[c] * P                     # node base per position
        for j in range(nbc):
            mu[blk[j]:blk[j] + P] = mo[j]
            ls[blk[j]:blk[j] + P] = lo[j]
    out = (mu[:N], ls[:N])
    if _want_results:
        return out, res
    return out



# revision 5
# speedup vs baseline: 1.3582x; 1.3582x over previous
"""GCN VGAE encoder (2-layer, mu/logstd heads) on 8 Trainium2 NeuronCores.

v2 strategy (edge-parallel over dst-sorted chunks, fp16 device math):
  - Host: sort edges by dst; bucket per (128-node dst block j, source quarter
    g4 = src//25088); pad each (j,g4) run to a uniform (across cores) chunk
    count of 128 edges.  Ships, per core:
      * z1   = x[src_e]           (edge-expanded L1 messages, pure reorder)
      * degs = deg[src_e]         (int16; device computes rsqrt(deg+1))
      * dstl = dst_e & 127        (fp16 one-hot targets; 128 => padding no-op)
      * idxw = int16 gather rows for the LAYER-2 table gather only
    Degrees come free from the CSR build.
  - Device (SPMD x8, each core owns 98 node blocks = 12544 nodes), fp16:
      L1: NO gather.  msgs1 = z1 * rsqrt(degs+1); one-hot via DVE is_equal
          (fp16); matmul-accumulate [2,128] sums in PSUM per dst block;
          self-loop via accumulating PE transpose; h = relu(disv*(s1@W1)+b1);
          y2 = disv*h (fp16)
      AllGather fat fp16 y2 table (rows padded to 128 cols = 256B so the
      dma_gather elem_size constraint is met)
      L2: dma_gather y2[src] per 128-edge chunk (4 SWDGE queues, elem 256B),
          one-hot matmul accumulation, self-loop via transpose, heads
          mu/logstd = disv*(s2@W) + b via small matmuls (f32 outputs).
All floating-point arithmetic runs on device; the host only reorders data.
"""
import math

import numpy as np

import concourse.bass as bass
import concourse.bacc as bacc
import concourse.mybir as mybir
import concourse.tile as tile
from concourse.bass_utils import run_bass_kernel_spmd
from concourse.masks import make_identity

P = 128
N_CORES = 8
NG = 4                      # source-quarter groups (int16 sub-tables)
FAT = 128                   # fp16 table row width (256B = gather elem floor)
F32 = mybir.dt.float32
F16 = mybir.dt.float16
I32 = mybir.dt.int32
I16 = mybir.dt.int16

_CACHE = {}


# ---------------------------------------------------------------- host prep
def _prep(x, edge_index):
    N = x.shape[0]
    in_ch = x.shape[1]
    nbc = math.ceil(math.ceil(N / N_CORES) / P)      # blocks per core (98)
    npc = nbc * P                                    # nodes per core (12544)
    npad = N_CORES * npc                             # padded nodes (100352)
    nblk = N_CORES * nbc                             # blocks (784)
    nsub = npad // NG                                # sub-table rows (25088)

    src = np.asarray(edge_index[0]).astype(np.int64)
    dst = np.asarray(edge_index[1]).astype(np.int64)
    E = src.shape[0]

    deg = np.bincount(dst, minlength=npad).astype(np.int32)

    c_ = src // npc
    g4 = c_ // 2                                      # source quarter

    # sort edges by (dst-block, g4, dst)
    order = np.argsort(((dst >> 7) * NG + g4) * 128 + (dst & 127),
                       kind="stable")
    dst_s = dst[order]
    src_s = src[order]

    cd = dst_s // npc
    j_s = (dst_s - cd * npc) >> 7                     # block pos within core
    run = (cd * nbc + j_s) * NG + g4[order]
    nrun = nblk * NG
    counts = np.bincount(run, minlength=nrun)
    rstart = np.zeros(nrun + 1, np.int64)
    np.cumsum(counts, out=rstart[1:])

    # chunks per (j, g4): max over cores (SPMD-uniform); sort blocks by size
    # per core so big blocks align with big blocks across cores.
    cnt = counts.reshape(N_CORES, nbc, NG)
    perm = np.argsort(-cnt.sum(axis=2), axis=1)                # [NC, nbc]
    pos = np.empty_like(perm)
    for c in range(N_CORES):
        pos[c, perm[c]] = np.arange(nbc)
    cntp = np.take_along_axis(cnt, perm[:, :, None], axis=1)
    S = np.ceil(cntp / P).astype(np.int64).max(axis=0)         # [nbc, NG]
    Tj = S.sum(axis=1)                                         # chunks per block
    off = np.zeros(nbc + 1, np.int64)
    np.cumsum(Tj, out=off[1:])
    T = int(off[-1])
    goff = np.zeros((nbc, NG), np.int64)
    goff[:, 1:] = np.cumsum(S[:, :-1], axis=1)
    Tg = S.sum(axis=0)                                         # chunks per g4
    cidx0 = np.zeros((nbc, NG), np.int64)
    cidx0[1:, :] = np.cumsum(S[:-1, :], axis=0)

    posg = pos.reshape(-1)                            # position of global blk
    cs = src_s // npc
    rems = src_s - cs * npc
    rows = cs * npc + (rems & 127) * nbc + posg[src_s >> 7]
    rloc_s = (rows - (cs // 2) * nsub).astype(np.int64)

    xf = np.asarray(x, np.float32)
    dstl = np.full((N_CORES, P, T), 128.0, np.float16)         # 128 => no-op
    z1 = np.zeros((N_CORES, P, T * in_ch), np.float32)
    degs = np.zeros((N_CORES, P, T), np.int16)
    idxw = [np.zeros((N_CORES, P, int(Tg[g]) * 8), np.int16) for g in range(NG)]

    for c in range(N_CORES):
        srcl_g = [np.zeros((P, int(Tg[g])), np.int16) for g in range(NG)]
        for j in range(nbc):
            for g in range(NG):
                r = (c * nbc + int(perm[c, j])) * NG + g
                e0, e1 = rstart[r], rstart[r + 1]
                n_e = e1 - e0
                if n_e == 0:
                    continue
                i = np.arange(n_e)
                lane = i & 127
                col = off[j] + goff[j, g] + (i >> 7)
                dstl[c, lane, col] = (dst_s[e0:e1] & 127).astype(np.float16)
                degs[c, lane, col] = deg[src_s[e0:e1]].astype(np.int16)
                for k in range(in_ch):
                    z1[c, lane, col * in_ch + k] = xf[src_s[e0:e1], k]
                ccol = cidx0[j, g] + (i >> 7)
                srcl_g[g][lane, ccol] = rloc_s[e0:e1].astype(np.int16)
        for g in range(NG):
            flat = srcl_g[g].T.ravel()                # i = chunk*128 + lane
            w16 = flat.reshape(-1, 16).T              # [16, Tg*8]
            idxw[g][c] = np.tile(w16, (8, 1))         # replicate to 128 parts

    # per-core own slices (node-space, permuted block order, partition-inner)
    gidx = np.concatenate([c * nbc + perm[c] for c in range(N_CORES)])
    xpad = np.zeros((npad, in_ch), np.float32)
    xpad[:N] = xf
    x_g = xpad.reshape(nblk, P, in_ch)[gidx].transpose(1, 0, 2).reshape(P, -1)
    deg_g = deg.reshape(nblk, P)[gidx].T
    x_own = np.stack([
        x_g[:, c * nbc * in_ch:(c + 1) * nbc * in_ch] for c in range(N_CORES)])
    deg_own = np.stack(
        [np.ascontiguousarray(deg_g[:, c * nbc:(c + 1) * nbc])
         for c in range(N_CORES)])

    iota = np.tile(np.arange(P, dtype=np.float16), (P, 1))

    meta = dict(N=N, E=E, in_ch=in_ch, nbc=nbc, npc=npc, npad=npad,
                nblk=nblk, nsub=nsub, T=T,
                S=tuple(map(tuple, S.tolist())), perm=perm,
                off=off, goff=goff, cidx0=cidx0, Tg=tuple(int(t) for t in Tg))
    arrays = dict(dstl=dstl, z1=z1, degs=degs, idxw=idxw,
                  x_own=x_own, deg_own=deg_own, iota=iota)
    return meta, arrays


# ---------------------------------------------------------------- device build
def _build(meta, in_ch, hid, out_ch):
    nbc, T = meta["nbc"], meta["T"]
    npc, nsub = meta["npc"], meta["nsub"]
    S, off, goff, cidx0, Tg = (meta["S"], meta["off"], meta["goff"],
                               meta["cidx0"], meta["Tg"])
    SPC = 14                 # chunks per gather instruction

    nc = bacc.Bacc("TRN2", target_bir_lowering=False, debug=False,
                   num_devices=N_CORES, num_swdge_queues=NG)

    z1_d = nc.dram_tensor("z1", [P, T * in_ch], F32, kind="ExternalInput")
    degs_d = nc.dram_tensor("degs", [P, T], I16, kind="ExternalInput")
    dstl_d = nc.dram_tensor("dstl", [P, T], F16, kind="ExternalInput")
    x_o_d = nc.dram_tensor("x_own", [P, nbc * in_ch], F32, kind="ExternalInput")
    deg_o_d = nc.dram_tensor("deg_own", [P, nbc], I32, kind="ExternalInput")
    idxw_d = [nc.dram_tensor(f"idxw{g}", [P, Tg[g] * 8], I16,
                             kind="ExternalInput") for g in range(NG)]
    iota_d = nc.dram_tensor("iota", [P, P], F16, kind="ExternalInput")
    w1_d = nc.dram_tensor("w1", [in_ch, hid], F32, kind="ExternalInput")
    b1_d = nc.dram_tensor("b1", [P, hid], F32, kind="ExternalInput")
    wmu_d = nc.dram_tensor("wmu", [hid, out_ch], F32, kind="ExternalInput")
    wls_d = nc.dram_tensor("wls", [hid, out_ch], F32, kind="ExternalInput")
    bmu_d = nc.dram_tensor("bmu", [P, out_ch], F32, kind="ExternalInput")
    bls_d = nc.dram_tensor("bls", [P, out_ch], F32, kind="ExternalInput")
    mu_o = nc.dram_tensor("mu_o", [P, nbc * out_ch], F32, kind="ExternalOutput")
    ls_o = nc.dram_tensor("ls_o", [P, nbc * out_ch], F32, kind="ExternalOutput")

    cc_in = nc.dram_tensor("cc_in", [npc, FAT], F16, kind="Internal")
    cc_out = nc.dram_tensor("cc_out", [N_CORES * npc, FAT], F16,
                            kind="Internal", addr_space="Shared")

    # gather instruction spans per g4: list of (c0, n) chunk ranges
    spans = []
    for g in range(NG):
        sp = []
        for c0 in range(0, Tg[g], SPC):
            sp.append((c0, min(SPC, Tg[g] - c0)))
        spans.append(sp)
    order1 = [(g, sp) for i in range(max(len(s) for s in spans))
              for g in range(NG) for sp in spans[g][i:i + 1]]

    with tile.TileContext(nc) as tc:
        with tc.tile_pool(name="const", bufs=1) as cp:
            z1_t = cp.tile([P, T * in_ch], F32)
            degs_t = cp.tile([P, T], I16)
            dstl_t = cp.tile([P, T], F16)
            idxw_t = [cp.tile([P, Tg[g] * 8], I16, name=f"idxw{g}_t")
                      for g in range(NG)]
            iota_t = cp.tile([P, P], F16)
            x_ot = cp.tile([P, nbc * in_ch], F32)
            deg_ot = cp.tile([P, nbc], I32)
            w1_t = cp.tile([in_ch, hid], F32)
            b1_t = cp.tile([P, hid], F32)
            wmu_t = cp.tile([hid, out_ch], F32)
            wls_t = cp.tile([hid, out_ch], F32)
            bmu_t = cp.tile([P, out_ch], F32)
            bls_t = cp.tile([P, out_ch], F32)
            w1_16 = cp.tile([in_ch, hid], F16)
            b1_16 = cp.tile([P, hid], F16)
            wmu_16 = cp.tile([hid, out_ch], F16)
            wls_16 = cp.tile([hid, out_ch], F16)
            ident = cp.tile([P, P], F32)
            disv_e = cp.tile([P, T], F32)
            msgs1 = cp.tile([P, T * in_ch], F16)
            disv_o = cp.tile([P, nbc], F32)
            y1_own = cp.tile([P, nbc * in_ch], F32)
            y2_t = cp.tile([P, nbc * hid], F32)
            stage = cp.tile([P, nbc * FAT], F16)
            mu_sb = cp.tile([P, nbc * out_ch], F32)
            ls_sb = cp.tile([P, nbc * out_ch], F32)

            for dt_, tt in ((z1_d, z1_t), (degs_d, degs_t), (dstl_d, dstl_t),
                            (x_o_d, x_ot), (deg_o_d, deg_ot),
                            (iota_d, iota_t), (w1_d, w1_t), (b1_d, b1_t),
                            (wmu_d, wmu_t), (wls_d, wls_t), (bmu_d, bmu_t),
                            (bls_d, bls_t)):
                nc.sync.dma_start(out=tt[:], in_=dt_[:])
            for g in range(NG):
                nc.sync.dma_start(out=idxw_t[g][:], in_=idxw_d[g][:])
            make_identity(nc, ident[:])
            for ft, st in ((w1_t, w1_16), (b1_t, b1_16),
                           (wmu_t, wmu_16), (wls_t, wls_16)):
                nc.vector.tensor_copy(out=st[:], in_=ft[:])

            # disv_e = 1/sqrt(degs+1) (per edge); disv_o per own node
            nc.vector.tensor_copy(out=disv_e[:], in_=degs_t[:])
            nc.scalar.activation(disv_e[:], disv_e[:],
                                 mybir.ActivationFunctionType.Sqrt, bias=1.0)
            nc.vector.reciprocal(out=disv_e[:], in_=disv_e[:])
            nc.vector.tensor_copy(out=disv_o[:], in_=deg_ot[:])
            nc.scalar.activation(disv_o[:], disv_o[:],
                                 mybir.ActivationFunctionType.Sqrt, bias=1.0)
            nc.vector.reciprocal(out=disv_o[:], in_=disv_o[:])

            # msgs1 = z1 * disv_e  (fp16);  y1_own = x_own * disv_o (fp16)
            nc.vector.tensor_tensor(
                out=msgs1[:].rearrange("p (t c) -> p t c", c=in_ch),
                in0=z1_t[:].rearrange("p (t c) -> p t c", c=in_ch),
                in1=disv_e[:].rearrange("p (t o) -> p t o", o=1)
                    .to_broadcast([P, T, in_ch]),
                op=mybir.AluOpType.mult)
            nc.vector.tensor_tensor(
                out=y1_own[:].rearrange("p (b c) -> p b c", c=in_ch),
                in0=x_ot[:].rearrange("p (b c) -> p b c", c=in_ch),
                in1=disv_o[:].rearrange("p (b o) -> p b o", o=1)
                    .to_broadcast([P, nbc, in_ch]),
                op=mybir.AluOpType.mult)

            # zero the fat staging columns once (only cols 0:hid get written)
            nc.vector.memset(stage[:], 0.0)

            # ---------------- Layer 1 (no gather) ----------------
            with (
                tc.tile_pool(name="oh1", bufs=3) as ohp,
                tc.tile_pool(name="s1", bufs=4) as s1p,
                tc.tile_pool(name="ps1", bufs=4, space="PSUM") as pp1,
                tc.tile_pool(name="psh", bufs=4, space="PSUM") as pph,
            ):
                for j in range(nbc):
                    tj = int(off[j + 1] - off[j])
                    c0 = int(off[j])
                    oh = ohp.tile([P, tj * P], F16, tag="oh")
                    nc.vector.tensor_tensor(
                        out=oh[:].rearrange("p (s f) -> p s f", f=P),
                        in0=iota_t[:].rearrange("p (o f) -> p o f", o=1)
                            .to_broadcast([P, tj, P]),
                        in1=dstl_t[:, c0:c0 + tj]
                            .rearrange("p (s o) -> p s o", o=1)
                            .to_broadcast([P, tj, P]),
                        op=mybir.AluOpType.is_equal)
                    ps1 = pp1.tile([in_ch, P], F32, tag="ps1")
                    first = True
                    for t in range(tj):
                        col = c0 + t
                        nc.tensor.matmul(
                            out=ps1[:],
                            lhsT=msgs1[:, col * in_ch:(col + 1) * in_ch],
                            rhs=oh[:, t * P:(t + 1) * P],
                            start=first, stop=False)
                        first = False
                    nc.tensor.matmul(
                        out=ps1[:], lhsT=y1_own[:, j * in_ch:(j + 1) * in_ch],
                        rhs=ident[:], is_transpose=True,
                        start=first, stop=True)
                    s1T = s1p.tile([in_ch, P], F16, tag="s1T")
                    nc.vector.tensor_copy(out=s1T[:], in_=ps1[:])
                    ph = pph.tile([P, hid], F32, tag="ph")
                    nc.tensor.matmul(out=ph[:], lhsT=s1T[:], rhs=w1_16[:],
                                     start=True, stop=True)
                    nc.vector.tensor_tensor(
                        out=y2_t[:, j * hid:(j + 1) * hid],
                        in0=ph[:],
                        in1=disv_o[:, j:j + 1].to_broadcast([P, hid]),
                        op=mybir.AluOpType.mult)

            # h = relu(. + b1); y2 = disv * h  (fp16)
            y2_3 = y2_t[:].rearrange("p (b c) -> p b c", c=hid)
            nc.vector.tensor_tensor(
                out=y2_3, in0=y2_3,
                in1=b1_16[:].rearrange("p (o c) -> p o c", o=1)
                    .to_broadcast([P, nbc, hid]),
                op=mybir.AluOpType.add)
            nc.scalar.activation(y2_t[:], y2_t[:],
                                 mybir.ActivationFunctionType.Relu)
            nc.vector.tensor_tensor(
                out=y2_3, in0=y2_3,
                in1=disv_o[:].rearrange("p (b o) -> p b o", o=1)
                    .to_broadcast([P, nbc, hid]),
                op=mybir.AluOpType.mult)

            # fat stage -> cc_in -> AllGather (fp16, 256B rows)
            nc.vector.tensor_copy(
                out=stage[:].rearrange("p (b w) -> p b w", w=FAT)[:, :, :hid],
                in_=y2_3)
            nc.sync.dma_start(
                out=cc_in.ap().rearrange("(p b) w -> p (b w)", p=P),
                in_=stage[:])
            nc.gpsimd.collective_compute(
                "AllGather", mybir.AluOpType.bypass,
                ins=[cc_in[:]], outs=[cc_out[:]],
                replica_groups=[list(range(N_CORES))])

            # ---------------- Layer 2 ----------------
            with (
                tc.tile_pool(name="g2", bufs=16) as g2p,
                tc.tile_pool(name="oh2", bufs=3) as ohp2,
                tc.tile_pool(name="s2", bufs=4) as s2p,
                tc.tile_pool(name="ps2", bufs=3, space="PSUM") as pp2,
                tc.tile_pool(name="pmu", bufs=2, space="PSUM") as ppm,
            ):
                gtiles2 = {g: [] for g in range(NG)}
                for g, (c0, n) in order1:
                        gt = g2p.tile([P, n * FAT], F16, tag="g2")
                        nc.gpsimd.dma_gather(
                            out_ap=gt[:].rearrange("p (n c) -> p n c", c=FAT),
                            in_ap=cc_out.ap()[g * nsub:(g + 1) * nsub, :],
                            idxs_ap=idxw_t[g][:, c0 * 8:(c0 + n) * 8],
                            num_idxs=n * P, num_idxs_reg=n * P,
                            elem_size=FAT, single_packet=False, queue_num=g)
                        gtiles2[g].append((c0, n, gt))

                def msg2(g, cidx):
                    for c0, n, gt in gtiles2[g]:
                        if c0 <= cidx < c0 + n:
                            return gt, cidx - c0
                    raise AssertionError

                for j in range(nbc):
                    tj = int(off[j + 1] - off[j])
                    c0 = int(off[j])
                    oh = ohp2.tile([P, tj * P], F16, tag="oh2")
                    nc.vector.tensor_tensor(
                        out=oh[:].rearrange("p (s f) -> p s f", f=P),
                        in0=iota_t[:].rearrange("p (o f) -> p o f", o=1)
                            .to_broadcast([P, tj, P]),
                        in1=dstl_t[:, c0:c0 + tj]
                            .rearrange("p (s o) -> p s o", o=1)
                            .to_broadcast([P, tj, P]),
                        op=mybir.AluOpType.is_equal)
                    ps2 = pp2.tile([hid, P], F32, tag="ps2")
                    first = True
                    for g in range(NG):
                        for t in range(S[j][g]):
                            gt, k = msg2(g, int(cidx0[j][g]) + t)
                            col = int(goff[j][g]) + t
                            nc.tensor.matmul(
                                out=ps2[:],
                                lhsT=gt[:, k * FAT:k * FAT + hid],
                                rhs=oh[:, col * P:(col + 1) * P],
                                start=first, stop=False)
                            first = False
                    nc.tensor.matmul(
                        out=ps2[:], lhsT=y2_t[:, j * hid:(j + 1) * hid],
                        rhs=ident[:], is_transpose=True,
                        start=first, stop=True)
                    s2T = s2p.tile([hid, P], F16, tag="s2T")
                    nc.vector.tensor_copy(out=s2T[:], in_=ps2[:])
                    pm = ppm.tile([P, 2 * out_ch], F32, tag="pm")
                    nc.tensor.matmul(out=pm[:, :out_ch], lhsT=s2T[:],
                                     rhs=wmu_16[:], start=True, stop=True)
                    nc.tensor.matmul(out=pm[:, out_ch:], lhsT=s2T[:],
                                     rhs=wls_16[:], start=True, stop=True)
                    nc.scalar.activation(
                        mu_sb[:, j * out_ch:(j + 1) * out_ch], pm[:, :out_ch],
                        mybir.ActivationFunctionType.Copy,
                        scale=disv_o[:, j:j + 1])
                    nc.scalar.activation(
                        ls_sb[:, j * out_ch:(j + 1) * out_ch], pm[:, out_ch:],
                        mybir.ActivationFunctionType.Copy,
                        scale=disv_o[:, j:j + 1])

            for sb, bt, ot in ((mu_sb, bmu_t, mu_o), (ls_sb, bls_t, ls_o)):
                nc.vector.tensor_tensor(
                    out=sb[:].rearrange("p (b c) -> p b c", c=out_ch),
                    in0=sb[:].rearrange("p (b c) -> p b c", c=out_ch),
                    in1=bt[:].rearrange("p (o c) -> p o c", o=1)
                        .to_broadcast([P, nbc, out_ch]),
                    op=mybir.AluOpType.add)
                nc.sync.dma_start(out=ot[:], in_=sb[:])

    nc.compile()
    return nc


# ---------------------------------------------------------------- entry point
def kernel(x, edge_index, W1, b1, W_mu, b_mu, W_logstd, b_logstd,
           _want_results=False, _run_kwargs=None):
    x = np.asarray(x, np.float32)
    in_ch, hid, out_ch = W1.shape[0], W1.shape[1], W_mu.shape[1]
    meta, arrays = _prep(x, edge_index)

    key = (meta["N"], meta["E"], in_ch, hid, out_ch, meta["S"])
    if key not in _CACHE:
        _CACHE[key] = _build(meta, in_ch, hid, out_ch)
    nc = _CACHE[key]

    b1_b = np.tile(np.asarray(b1, np.float32), (P, 1))
    bmu_b = np.tile(np.asarray(b_mu, np.float32), (P, 1))
    bls_b = np.tile(np.asarray(b_logstd, np.float32), (P, 1))

    in_maps = []
    for c in range(N_CORES):
        m = dict(
            z1=arrays["z1"][c], degs=arrays["degs"][c],
            dstl=arrays["dstl"][c],
            x_own=arrays["x_own"][c], deg_own=arrays["deg_own"][c],
            iota=arrays["iota"],
            w1=np.asarray(W1, np.float32), b1=b1_b,
            wmu=np.asarray(W_mu, np.float32), wls=np.asarray(W_logstd, np.float32),
            bmu=bmu_b, bls=bls_b)
        for g in range(NG):
            m[f"idxw{g}"] = arrays["idxw"][g][c]
        in_maps.append(m)

    res = run_bass_kernel_spmd(nc, in_maps, core_ids=list(range(N_CORES)),
                               **(_run_kwargs or {}))

    N, nbc, npc = meta["N"], meta["nbc"], meta["npc"]
    perm = meta["perm"]
    mu = np.empty((N_CORES * npc, out_ch), np.float32)
    ls = np.empty((N_CORES * npc, out_ch), np.float32)
    for c in range(N_CORES):
        mo = res.results[c]["mu_o"].reshape(P, nbc, out_ch).transpose(1, 0, 2)
        lo = res.results[c]["ls_o"].reshape(P, nbc, out_ch).transpose(1, 0, 2)
        blk = c * npc + perm[c] * P                     # node base per position
        for j in range(nbc):
            mu[blk[j]:blk[j] + P] = mo[j]
            ls[blk[j]:blk[j] + P] = lo[j]
    out = (mu[:N], ls[:N])
    if _want_results:
        return out, res
    return out
